# revision 21
# baseline (speedup 1.0000x reference)
"""12-qubit quantum layer on 8 NeuronCores — layered kron kernel.

Math: the circuit is encode -> [Rot layer L (kron A_L x B_L over hi/lo
6-qubit halves); masked CNOT ring L] for L=0..3.  Host folds encode+Rot0+
ring0 into the uploaded product states, and ring3 into the sign matrix.
The device applies layers 1..3 as 64x64 block matmuls on the (hi, lo)
state matrix with rings 1,2 folded in:
  - hi/lo-only CNOTs and ring perms fold into the 128x128 stationary
    operands (I2 x W blockdiag over a 2-sample partition pairing);
  - the single boundary-crossing CNOT per ring becomes a column-split
    matmul (two stationary variants over contiguous free-column halves,
    after a free relabeling that moves the control set to the top bit).

Device pipeline per core (32 samples, bf16, fp32 PSUM), batch in 2
pipelined groups of 16: P1(lo) T1 P2(hi) P3(hi,split) T2 P4(lo)
P5(lo,split) T3 P6(hi); transposes are PE block-transposes.  DMA is
~1.8 MB/core/rep (vs 9.7 MB for the dense-matmul formulation).
"""
import os
import numpy as np
import ml_dtypes

import concourse.bass as bass
import concourse.bacc as bacc
import concourse.mybir as mybir
import concourse.tile as tile
from concourse.bass_utils import run_bass_kernel_spmd

N = 12
DIM = 1 << N                 # 4096
BATCH = 256
N_CORES = 8
BPC = BATCH // N_CORES       # 32 samples per core
GROUPS = [(0, 8), (8, 16)]             # j16 column ranges per pipeline group
NG = len(GROUPS)

BF16 = mybir.dt.bfloat16
F32 = mybir.dt.float32

LAST_EXEC_NS = None
LAST_RESULTS = None
_NC_CACHE = {}

# weight slot indices in w tensor [128, 19, 128]
_W_F1, _W_P23A, _W_P23B, _W_P45A, _W_P45B, _W_P6 = 0, 3, 6, 9, 12, 15
_W_ID = 18
_NW = 19


# ---------------- host circuit algebra ----------------

def _rot(phi, th, om):
    def rz(t):
        e = np.exp(-1j * t / 2)
        return np.array([[e, 0], [0, np.conj(e)]], dtype=np.complex128)
    c, s = np.cos(th / 2), np.sin(th / 2)
    ry = np.array([[c, -s], [s, c]], dtype=np.complex128)
    return rz(om) @ ry @ rz(phi)


def _kron6(gs):
    K = gs[0]
    for g in gs[1:]:
        K = np.kron(K, g)
    return K


def _cnot_perm(c, t):
    idx = np.arange(DIM)
    return idx ^ (((idx >> (N - 1 - c)) & 1) << (N - 1 - t))


def _ring_perm(mask, L):
    perm = np.arange(DIM)
    for q in range(N):
        if mask[L, q]:
            perm = perm[_cnot_perm(q, (q + 1) % N)]
    return perm


def _half_perm(mask, L, lo):
    """Perm over 64 labels from hi-only (q 0..4) or lo-only (q 6..10) CNOTs."""
    perm = np.arange(64)
    rng = range(6, 11) if lo else range(0, 5)
    base = 11 if lo else 5
    for q in rng:
        if mask[L, q]:
            cpos, tpos = base - q, base - (q + 1)
            idx = np.arange(64)
            perm = perm[idx ^ (((idx >> cpos) & 1) << tpos)]
    return perm


def _stage_matrices(weights, entanglers):
    """The 8 stage matrices (with ring folds + relabelings) and host maps."""
    mask = np.asarray(entanglers) > 0.5
    w = np.asarray(weights, dtype=np.float64)
    assert not mask[1, 5], "ring1 X5 crossing unsupported by fold schedule"
    assert not mask[2, 11], "ring2 X11 crossing unsupported by fold schedule"

    fac = {}
    for L in (1, 2, 3):
        gs = [_rot(*w[L, q]) for q in range(N)]
        fac[L] = (_kron6(gs[:6]), _kron6(gs[6:]))
    A1, B1 = fac[1]
    A2, B2 = fac[2]
    A3, B3 = fac[3]

    ph1, pl1 = _half_perm(mask, 1, 0), _half_perm(mask, 1, 1)
    ph2, pl2 = _half_perm(mask, 2, 0), _half_perm(mask, 2, 1)
    I = np.eye(64)
    Ip = lambda p: I[p]
    xor32 = np.arange(64) ^ 32

    # ring1 X11 crossing (ctrl odd lo cols, rows hi^32), commuted before
    # loP(pl1) and hiP(ph1):
    if mask[1, 11]:
        c_a = np.zeros(64, dtype=bool)
        c_a[pl1[np.arange(64) % 2 == 1]] = True
        ph1inv = np.argsort(ph1)
        tau = np.array([ph1[ph1inv[g] ^ 32] for g in range(64)])
    else:
        c_a = np.zeros(64, dtype=bool)
        c_a[32:] = True                       # degenerate split, W3a == W3b
        tau = np.arange(64)
    # ring2 X5 crossing (ctrl odd hi rows, cols lo^32), commuted before ph2:
    if mask[2, 5]:
        r_c = np.zeros(64, dtype=bool)
        r_c[ph2[np.arange(64) % 2 == 1]] = True
        x5map = xor32
    else:
        r_c = np.zeros(64, dtype=bool)
        r_c[32:] = True
        x5map = np.arange(64)

    W1 = B1
    W2 = A1
    W3a = A2 @ Ip(ph1)
    W3b = A2 @ Ip(ph1) @ Ip(tau)
    W4 = B2 @ Ip(pl1)
    W5a = B3 @ Ip(pl2)
    W5b = B3 @ Ip(pl2) @ Ip(x5map)
    W6 = A3 @ Ip(ph2)

    def relabel(ctrl):
        r = np.empty(64, dtype=np.int64)
        r[~ctrl] = np.arange(32)
        r[ctrl] = 32 + np.arange(32)
        return r

    rho = relabel(c_a)
    sig = relabel(r_c)
    W1 = Ip(np.argsort(rho)) @ W1
    W3a = Ip(np.argsort(sig)) @ W3a
    W3b = Ip(np.argsort(sig)) @ W3b
    W4 = W4 @ Ip(rho)
    W6 = W6 @ Ip(sig)
    # merged stages: P23 = P3 o P2 (hi), P45 = P5 o P4 (lo)
    return [W1, W3a @ W2, W3b @ W2, W5a @ W4, W5b @ W4, W6], mask


def _encode_psi2(x, weights, entanglers):
    """Product states through encode + Rot layer 0, then ring0 perm."""
    mask = np.asarray(entanglers) > 0.5
    x64 = np.asarray(x, dtype=np.float64)
    pi = np.pi
    c = np.cos(x64 * pi / 2)
    s = np.sin(x64 * pi / 2)
    ph = np.exp(-1j * (x64 ** 2) * pi / 2)
    psi = np.ones((x64.shape[0], 1), np.complex128)
    for q in range(N):
        v = np.stack([ph[:, q] * c[:, q], np.conj(ph[:, q]) * s[:, q]], axis=-1)
        v = v @ _rot(*np.asarray(weights, np.float64)[0, q]).T
        psi = (psi[:, :, None] * v[:, None, :]).reshape(x64.shape[0], -1)
    return psi[:, _ring_perm(mask, 0)]


def _bd(W):
    """blockdiag(I2 x W) as lhsT [128,128]: out = W @ S per j2 block."""
    Z = np.zeros((128, 128), np.float64)
    Z[:64, :64] = W.T
    Z[64:, 64:] = W.T
    return Z


def _pack_w(Ws):
    """19 [128,128] mats -> [128, 19, 128] packed.

    Slot 0-2 (F1) are rhs operands of the fused transpose-matmuls
    (out_blk = S_blk.T @ bd(W1)); the rest are lhsT stationary operands.
    Both use _bd(): as lhsT it applies blockdiag(W) on partitions, as
    fused rhs it right-multiplies by blockdiag(W).T."""
    W1, W23a, W23b, W45a, W45b, W6 = Ws
    mats = []
    for W in (W1, W23a, W23b, W45a, W45b, W6):
        mats += [_bd(W.real), _bd(-W.imag), _bd(W.imag)]
    mats.append(np.eye(128))
    arr = np.stack(mats).astype(np.float32)          # [19, 128, 128]
    return np.ascontiguousarray(arr.transpose(1, 0, 2))


def _pack_psi(psi_core):
    """(32, 4096) complex -> [2, 128, 16, 64] float32 planes, layout B."""
    S = psi_core.reshape(2, 16, 64, 64)              # (j2, j16, hi, lo)
    P = S.transpose(0, 3, 1, 2).reshape(128, 16, 64)  # (j2*64+lo, j16, hi)
    return np.stack([P.real, P.imag]).astype(np.float32)


def _unpack_slot():
    """slot[g2, g16] = original sample index, after 3 transpose shuffles."""
    slot = np.arange(BPC).reshape(2, 16)
    for _ in range(3):
        new = np.empty_like(slot)
        for j2 in range(2):
            for j16 in range(16):
                k, m = j16 // 2, j16 % 2
                new[m, 2 * k + j2] = slot[j2, j16]
        slot = new
    return slot


# ---------------- device kernel ----------------

def _build_nc(repeats=1):
    nc = bacc.Bacc("TRN2", target_bir_lowering=False, debug=False)
    p_d = nc.dram_tensor("p", [2, 128, 16, 64], BF16, kind="ExternalInput")
    w_d = nc.dram_tensor("w", [128, _NW, 128], BF16, kind="ExternalInput")
    o_d = nc.dram_tensor("o", [2, 128, 16, 64], BF16, kind="ExternalOutput")
    NST = 6                       # pipeline stages
    with tile.TileContext(nc) as tc:
        with (
            tc.tile_pool(name="wp", bufs=1) as wp,
            tc.tile_pool(name="sp", bufs=1) as sp,
            tc.tile_pool(name="ps", bufs=1, space=bass.MemorySpace.PSUM) as ps,
        ):
            # weights are rep-invariant: load once, keep resident
            wt = wp.tile([128, _NW, 128], BF16, name="wt")
            nc.scalar.dma_start(wt[:], w_d[:])
            ident = wt[:, _W_ID, :]

            # PE warm-up while the weight/psi DMAs land
            warm = sp.tile([128, 128], BF16, name="warm")
            nc.vector.memset(warm[:], 0)
            wps = ps.tile([128, 6, 64], F32, name="psr", bufs=3)
            for _ in range(10):
                nc.tensor.matmul(wps[:, 0:2, :], warm[:], warm[:],
                                 start=True, stop=True, skip_group_check=True)

            def mm_stage(widx, src, g):
                w_ = GROUPS[g][1] - GROUPS[g][0]
                pr = ps.tile([128, w_, 64], F32, name="psr", bufs=3)
                pi_ = ps.tile([128, w_, 64], F32, name="psi", bufs=3)
                wr = wt[:, widx, :]
                wni = wt[:, widx + 1, :]
                wi = wt[:, widx + 2, :]
                sr, si = src
                nc.tensor.matmul(pr[:], wr, sr[:], start=True, stop=False,
                                 skip_group_check=True)
                nc.tensor.matmul(pi_[:], wr, si[:], start=True, stop=False,
                                 skip_group_check=True)
                nc.tensor.matmul(pr[:], wni, si[:], start=False, stop=True,
                                 skip_group_check=True)
                nc.tensor.matmul(pi_[:], wi, sr[:], start=False, stop=True,
                                 skip_group_check=True)
                return pr, pi_

            def mm_split(wa, wb, src, g):
                w_ = GROUPS[g][1] - GROUPS[g][0]
                pr = ps.tile([128, w_, 64], F32, name="psr", bufs=3)
                pi_ = ps.tile([128, w_, 64], F32, name="psi", bufs=3)
                sr, si = src
                for widx, hs in ((wa, slice(0, 32)), (wb, slice(32, 64))):
                    wr = wt[:, widx, :]
                    wni = wt[:, widx + 1, :]
                    wi = wt[:, widx + 2, :]
                    nc.tensor.matmul(pr[:, :, hs], wr, sr[:, :, hs],
                                     start=True, stop=False,
                                     skip_group_check=True)
                    nc.tensor.matmul(pi_[:, :, hs], wr, si[:, :, hs],
                                     start=True, stop=False,
                                     skip_group_check=True)
                    nc.tensor.matmul(pr[:, :, hs], wni, si[:, :, hs],
                                     start=False, stop=True,
                                     skip_group_check=True)
                    nc.tensor.matmul(pi_[:, :, hs], wi, sr[:, :, hs],
                                     start=False, stop=True,
                                     skip_group_check=True)
                return pr, pi_

            def fused_stage(widx, src, g):
                """fused lo-matmul + transpose: out_blk = S_blk.T @ bd(W);
                regular matmuls with the state block as stationary, ordered
                to share each stationary load across two matmuls."""
                w_ = GROUPS[g][1] - GROUPS[g][0]
                pr = ps.tile([128, w_, 64], F32, name="psr", bufs=3)
                pi_ = ps.tile([128, w_, 64], F32, name="psi", bufs=3)
                rr = wt[:, widx, :]
                rni = wt[:, widx + 1, :]
                ri = wt[:, widx + 2, :]
                sr, si = src
                for k in range(w_ // 2):
                    bs = slice(2 * k, 2 * k + 2)
                    nc.tensor.matmul(pr[:, bs, :], sr[:, bs, :], rr,
                                     start=True, stop=False,
                                     skip_group_check=True)
                    nc.tensor.matmul(pi_[:, bs, :], sr[:, bs, :], ri,
                                     start=True, stop=False,
                                     skip_group_check=True)
                    nc.tensor.matmul(pi_[:, bs, :], si[:, bs, :], rr,
                                     start=False, stop=True,
                                     skip_group_check=True)
                    nc.tensor.matmul(pr[:, bs, :], si[:, bs, :], rni,
                                     start=False, stop=True,
                                     skip_group_check=True)
                return pr, pi_

            def tr_stage(src, g):
                w_ = GROUPS[g][1] - GROUPS[g][0]
                pr = ps.tile([128, w_, 64], BF16, name="ptr", bufs=1)
                pi_ = ps.tile([128, w_, 64], BF16, name="pti", bufs=1)
                sr, si = src
                for k in range(w_ // 2):
                    bs = slice(2 * k, 2 * k + 2)
                    nc.tensor.transpose(pr[:, bs, :], sr[:, bs, :], ident)
                    nc.tensor.transpose(pi_[:, bs, :], si[:, bs, :], ident)
                return pr, pi_

            ev_engines = [
                lambda d, s: nc.vector.tensor_copy(d, s),
                lambda d, s: nc.scalar.copy(d, s),
            ]
            ev_ctr = [0]

            def evict(pr, pi_, name, g):
                w_ = GROUPS[g][1] - GROUPS[g][0]
                sr = sp.tile([128, w_, 64], BF16, name=f"s{name}r{g}", bufs=2)
                si = sp.tile([128, w_, 64], BF16, name=f"s{name}i{g}", bufs=2)
                ev_engines[ev_ctr[0] % 2](sr[:], pr[:])
                ev_engines[(ev_ctr[0] + 1) % 2](si[:], pi_[:])
                ev_ctr[0] += 1
                return sr, si

            plan = [
                ("fu", _W_F1, "1"),
                ("sp", (_W_P23A, _W_P23B), "23"),
                ("tr", None, "t2"),
                ("sp", (_W_P45A, _W_P45B), "45"),
                ("tr", None, "t3"),
                ("mm", _W_P6, "6"),
            ]

            def evict_pool(pr, pi_, name, g):
                w_ = GROUPS[g][1] - GROUPS[g][0]
                sr = sp.tile([128, w_, 64], BF16, name=f"s{name}r{g}", bufs=2)
                si = sp.tile([128, w_, 64], BF16, name=f"s{name}i{g}", bufs=2)
                nc.gpsimd.tensor_copy(sr[:], pr[:])
                nc.gpsimd.tensor_copy(si[:], pi_[:])
                return sr, si

            def emit_stage(s, src, g):
                kind, wi_, nm = plan[s]
                if kind == "mm":
                    pr, pi_ = mm_stage(wi_, src, g)
                elif kind == "sp":
                    pr, pi_ = mm_split(wi_[0], wi_[1], src, g)
                elif kind == "fu":
                    pr, pi_ = fused_stage(wi_, src, g)
                else:
                    pr, pi_ = tr_stage(src, g)
                return evict(pr, pi_, nm, g)

            # modulo-scheduled rep loop: at tick t, stage s runs rep t-s, so
            # adjacent engine-queue entries belong to independent chains and
            # every instruction's deps resolve before it reaches the head.
            cur = {}
            for tick in range(repeats + NST - 1):
                if tick < repeats:
                    pin = [[sp.tile([128, GROUPS[g][1] - GROUPS[g][0], 64],
                                    BF16, name=f"pin{pl}{g}", bufs=3)
                            for g in range(NG)] for pl in range(2)]
                    for g in range(NG):
                        a_, b_ = GROUPS[g]
                        for pl in range(2):
                            nc.sync.dma_start(pin[pl][g][:], p_d[pl][:, a_:b_, :])
                    for g in range(NG):
                        cur[(tick, g)] = (pin[0][g], pin[1][g])
                for s in range(NST):
                    rep = tick - s
                    if 0 <= rep < repeats:
                        for g in range(NG):
                            cur[(rep, g)] = emit_stage(s, cur[(rep, g)], g)
                rep_out = tick - (NST - 1)
                if rep_out >= 0:
                    for g in range(NG):
                        a_, b_ = GROUPS[g]
                        for pl in range(2):
                            nc.sync.dma_start(o_d[pl][:, a_:b_, :],
                                              cur[(rep_out, g)][pl][:])
                    for g in range(NG):
                        del cur[(rep_out, g)]
    nc.compile()
    return nc


# ---------------- host pre/post ----------------

def _prepare_in_maps(x, weights, entanglers):
    bf16 = ml_dtypes.bfloat16
    Ws, mask = _stage_matrices(weights, entanglers)
    W = _pack_w(Ws).astype(bf16)
    psi2 = _encode_psi2(x, weights, entanglers)
    in_maps = []
    for g in range(N_CORES):
        P = _pack_psi(psi2[BPC * g:BPC * (g + 1)]).astype(bf16)
        in_maps.append({"p": P, "w": W})
    return in_maps


def _postprocess(results, entanglers):
    mask = np.asarray(entanglers) > 0.5
    perm3 = _ring_perm(mask, 3)
    bits = (np.arange(DIM)[:, None] >> np.arange(N - 1, -1, -1)[None, :]) & 1
    signs = (1 - 2 * bits).astype(np.float32)
    slot = _unpack_slot()
    out = np.empty((BATCH, N), np.float32)
    for g in range(N_CORES):
        O = np.asarray(results[g]["o"], dtype=np.float32)  # [2, 128, 16, 64]
        P = O
        psi4 = np.empty((BPC, DIM), np.complex64)
        Pc = (P[0] + 1j * P[1]).reshape(2, 64, 16, 64)
        for g2 in range(2):
            for g16 in range(16):
                psi4[slot[g2, g16]] = Pc[g2, :, g16, :].reshape(DIM)
        psif = psi4[:, perm3]
        probs = (psif.real ** 2 + psif.imag ** 2).astype(np.float32)
        norm = probs.sum(axis=1, keepdims=True)
        out[BPC * g:BPC * (g + 1)] = (probs / norm) @ signs
    return out


def kernel(x, weights, entanglers):
    global LAST_EXEC_NS, LAST_RESULTS
    key = (np.asarray(x).tobytes(), np.asarray(weights).tobytes(),
           np.asarray(entanglers).tobytes())
    cached = _NC_CACHE.get("in_maps")
    if cached is not None and cached[0] == key:
        in_maps = cached[1]
    else:
        in_maps = _prepare_in_maps(x, weights, entanglers)
        _NC_CACHE["in_maps"] = (key, in_maps)

    if "nc" not in _NC_CACHE:
        _NC_CACHE["nc"] = _build_nc()
    nc = _NC_CACHE["nc"]

    trace = bool(os.environ.get("KERNEL_TRACE"))
    try:
        res = run_bass_kernel_spmd(nc, in_maps, core_ids=list(range(N_CORES)),
                                   trace=trace)
    except ModuleNotFoundError:
        res = run_bass_kernel_spmd(nc, in_maps, core_ids=list(range(N_CORES)),
                                   trace=False)
    LAST_RESULTS = res
    LAST_EXEC_NS = res.exec_time_ns
    return _postprocess(res.results, entanglers)


# revision 22
# speedup vs baseline: 1.7984x; 1.7984x over previous
"""12-qubit quantum layer on 8 NeuronCores — layered kron kernel.

Math: the circuit is encode -> [Rot layer L (kron A_L x B_L over hi/lo
6-qubit halves); masked CNOT ring L] for L=0..3.  Host folds encode+Rot0+
ring0 into the uploaded product states, and ring3 into the sign matrix.
The device applies layers 1..3 as 64x64 block matmuls on the (hi, lo)
state matrix with rings 1,2 folded in:
  - hi/lo-only CNOTs and ring perms fold into the 128x128 stationary
    operands (I2 x W blockdiag over a 2-sample partition pairing);
  - the single boundary-crossing CNOT per ring becomes a column-split
    matmul (two stationary variants over contiguous free-column halves,
    after a free relabeling that moves the control set to the top bit).

Device pipeline per core (32 samples, bf16 operands, fp32 PSUM), batch
split into 2 column-range groups, 6 PE stages:
  F1 (K1-lo fused INTO the B->A transpose: per 128-col block,
      out_blk = S_blk.T @ blockdiag(W1) via regular matmuls with the
      state block as the stationary operand — the lo-matmul rides the
      transpose for free), P23 (hi, col-split), T2, P45 (lo, col-split),
  T3, P6 (hi).
The in-NEFF repeat loop is modulo-scheduled (stage s of rep t-s at tick
t) so adjacent entries in each in-order engine queue belong to
independent chains and dependency waits resolve off the critical path.
Weights (19 x [128,128] bf16) load once; psi streams per rep.  DMA is
~1 MB/core/rep vs 9.7 MB for the dense-matmul formulation.
"""
import os
import numpy as np
import ml_dtypes

import concourse.bass as bass
import concourse.bacc as bacc
import concourse.mybir as mybir
import concourse.tile as tile
from concourse.bass_utils import run_bass_kernel_spmd

N = 12
DIM = 1 << N                 # 4096
BATCH = 256
N_CORES = 8
BPC = BATCH // N_CORES       # 32 samples per core
GROUPS = [(0, 8), (8, 16)]             # j16 column ranges per pipeline group
NG = len(GROUPS)

BF16 = mybir.dt.bfloat16
F32 = mybir.dt.float32

LAST_EXEC_NS = None
LAST_RESULTS = None
_NC_CACHE = {}

# weight slot indices in w tensor [128, 19, 128]
_W_F1, _W_P23A, _W_P23B, _W_P45A, _W_P45B, _W_P6 = 0, 3, 6, 9, 12, 15
_W_ID = 18
_NW = 19


# ---------------- host circuit algebra ----------------

def _rot(phi, th, om):
    def rz(t):
        e = np.exp(-1j * t / 2)
        return np.array([[e, 0], [0, np.conj(e)]], dtype=np.complex128)
    c, s = np.cos(th / 2), np.sin(th / 2)
    ry = np.array([[c, -s], [s, c]], dtype=np.complex128)
    return rz(om) @ ry @ rz(phi)


def _kron6(gs):
    K = gs[0]
    for g in gs[1:]:
        K = np.kron(K, g)
    return K


def _cnot_perm(c, t):
    idx = np.arange(DIM)
    return idx ^ (((idx >> (N - 1 - c)) & 1) << (N - 1 - t))


def _ring_perm(mask, L):
    perm = np.arange(DIM)
    for q in range(N):
        if mask[L, q]:
            perm = perm[_cnot_perm(q, (q + 1) % N)]
    return perm


def _half_perm(mask, L, lo):
    """Perm over 64 labels from hi-only (q 0..4) or lo-only (q 6..10) CNOTs."""
    perm = np.arange(64)
    rng = range(6, 11) if lo else range(0, 5)
    base = 11 if lo else 5
    for q in rng:
        if mask[L, q]:
            cpos, tpos = base - q, base - (q + 1)
            idx = np.arange(64)
            perm = perm[idx ^ (((idx >> cpos) & 1) << tpos)]
    return perm


def _stage_matrices(weights, entanglers):
    """The 8 stage matrices (with ring folds + relabelings) and host maps."""
    mask = np.asarray(entanglers) > 0.5
    w = np.asarray(weights, dtype=np.float64)
    assert not mask[1, 5], "ring1 X5 crossing unsupported by fold schedule"
    assert not mask[2, 11], "ring2 X11 crossing unsupported by fold schedule"

    fac = {}
    for L in (1, 2, 3):
        gs = [_rot(*w[L, q]) for q in range(N)]
        fac[L] = (_kron6(gs[:6]), _kron6(gs[6:]))
    A1, B1 = fac[1]
    A2, B2 = fac[2]
    A3, B3 = fac[3]

    ph1, pl1 = _half_perm(mask, 1, 0), _half_perm(mask, 1, 1)
    ph2, pl2 = _half_perm(mask, 2, 0), _half_perm(mask, 2, 1)
    I = np.eye(64)
    Ip = lambda p: I[p]
    xor32 = np.arange(64) ^ 32

    # ring1 X11 crossing (ctrl odd lo cols, rows hi^32), commuted before
    # loP(pl1) and hiP(ph1):
    if mask[1, 11]:
        c_a = np.zeros(64, dtype=bool)
        c_a[pl1[np.arange(64) % 2 == 1]] = True
        ph1inv = np.argsort(ph1)
        tau = np.array([ph1[ph1inv[g] ^ 32] for g in range(64)])
    else:
        c_a = np.zeros(64, dtype=bool)
        c_a[32:] = True                       # degenerate split, W3a == W3b
        tau = np.arange(64)
    # ring2 X5 crossing (ctrl odd hi rows, cols lo^32), commuted before ph2:
    if mask[2, 5]:
        r_c = np.zeros(64, dtype=bool)
        r_c[ph2[np.arange(64) % 2 == 1]] = True
        x5map = xor32
    else:
        r_c = np.zeros(64, dtype=bool)
        r_c[32:] = True
        x5map = np.arange(64)

    W1 = B1
    W2 = A1
    W3a = A2 @ Ip(ph1)
    W3b = A2 @ Ip(ph1) @ Ip(tau)
    W4 = B2 @ Ip(pl1)
    W5a = B3 @ Ip(pl2)
    W5b = B3 @ Ip(pl2) @ Ip(x5map)
    W6 = A3 @ Ip(ph2)

    def relabel(ctrl):
        r = np.empty(64, dtype=np.int64)
        r[~ctrl] = np.arange(32)
        r[ctrl] = 32 + np.arange(32)
        return r

    rho = relabel(c_a)
    sig = relabel(r_c)
    W1 = Ip(np.argsort(rho)) @ W1
    W3a = Ip(np.argsort(sig)) @ W3a
    W3b = Ip(np.argsort(sig)) @ W3b
    W4 = W4 @ Ip(rho)
    W6 = W6 @ Ip(sig)
    # merged stages: P23 = P3 o P2 (hi), P45 = P5 o P4 (lo)
    return [W1, W3a @ W2, W3b @ W2, W5a @ W4, W5b @ W4, W6], mask


def _encode_psi2(x, weights, entanglers):
    """Product states through encode + Rot layer 0, then ring0 perm."""
    mask = np.asarray(entanglers) > 0.5
    x64 = np.asarray(x, dtype=np.float64)
    pi = np.pi
    c = np.cos(x64 * pi / 2)
    s = np.sin(x64 * pi / 2)
    ph = np.exp(-1j * (x64 ** 2) * pi / 2)
    psi = np.ones((x64.shape[0], 1), np.complex128)
    for q in range(N):
        v = np.stack([ph[:, q] * c[:, q], np.conj(ph[:, q]) * s[:, q]], axis=-1)
        v = v @ _rot(*np.asarray(weights, np.float64)[0, q]).T
        psi = (psi[:, :, None] * v[:, None, :]).reshape(x64.shape[0], -1)
    return psi[:, _ring_perm(mask, 0)]


def _bd(W):
    """blockdiag(I2 x W) as lhsT [128,128]: out = W @ S per j2 block."""
    Z = np.zeros((128, 128), np.float64)
    Z[:64, :64] = W.T
    Z[64:, 64:] = W.T
    return Z


def _pack_w(Ws):
    """19 [128,128] mats -> [128, 19, 128] packed.

    Slot 0-2 (F1) are rhs operands of the fused transpose-matmuls
    (out_blk = S_blk.T @ bd(W1)); the rest are lhsT stationary operands.
    Both use _bd(): as lhsT it applies blockdiag(W) on partitions, as
    fused rhs it right-multiplies by blockdiag(W).T."""
    W1, W23a, W23b, W45a, W45b, W6 = Ws
    mats = []
    for W in (W1, W23a, W23b, W45a, W45b, W6):
        mats += [_bd(W.real), _bd(-W.imag), _bd(W.imag)]
    mats.append(np.eye(128))
    arr = np.stack(mats).astype(np.float32)          # [19, 128, 128]
    return np.ascontiguousarray(arr.transpose(1, 0, 2))


def _pack_psi(psi_core):
    """(32, 4096) complex -> [2, 128, 16, 64] float32 planes, layout B."""
    S = psi_core.reshape(2, 16, 64, 64)              # (j2, j16, hi, lo)
    P = S.transpose(0, 3, 1, 2).reshape(128, 16, 64)  # (j2*64+lo, j16, hi)
    return np.stack([P.real, P.imag]).astype(np.float32)


def _unpack_slot():
    """slot[g2, g16] = original sample index, after 3 transpose shuffles."""
    slot = np.arange(BPC).reshape(2, 16)
    for _ in range(3):
        new = np.empty_like(slot)
        for j2 in range(2):
            for j16 in range(16):
                k, m = j16 // 2, j16 % 2
                new[m, 2 * k + j2] = slot[j2, j16]
        slot = new
    return slot


# ---------------- device kernel ----------------

def _build_nc(repeats=1):
    nc = bacc.Bacc("TRN2", target_bir_lowering=False, debug=False)
    p_d = nc.dram_tensor("p", [2, 128, 16, 64], BF16, kind="ExternalInput")
    w_d = nc.dram_tensor("w", [128, _NW, 128], BF16, kind="ExternalInput")
    o_d = nc.dram_tensor("o", [2, 128, 16, 64], BF16, kind="ExternalOutput")
    NST = 6                       # pipeline stages
    with tile.TileContext(nc) as tc:
        with (
            tc.tile_pool(name="wp", bufs=1) as wp,
            tc.tile_pool(name="sp", bufs=1) as sp,
            tc.tile_pool(name="ps", bufs=1, space=bass.MemorySpace.PSUM) as ps,
        ):
            # weights are rep-invariant: load once, keep resident
            wt = wp.tile([128, _NW, 128], BF16, name="wt")
            nc.scalar.dma_start(wt[:], w_d[:])
            ident = wt[:, _W_ID, :]

            # PE warm-up while the weight/psi DMAs land
            warm = sp.tile([128, 128], BF16, name="warm")
            nc.vector.memset(warm[:], 0)
            wps = ps.tile([128, 6, 64], F32, name="psr", bufs=3)
            for _ in range(10):
                nc.tensor.matmul(wps[:, 0:2, :], warm[:], warm[:],
                                 start=True, stop=True, skip_group_check=True)

            def mm_stage(widx, src, g):
                w_ = GROUPS[g][1] - GROUPS[g][0]
                pr = ps.tile([128, w_, 64], F32, name="psr", bufs=3)
                pi_ = ps.tile([128, w_, 64], F32, name="psi", bufs=3)
                wr = wt[:, widx, :]
                wni = wt[:, widx + 1, :]
                wi = wt[:, widx + 2, :]
                sr, si = src
                nc.tensor.matmul(pr[:], wr, sr[:], start=True, stop=False,
                                 skip_group_check=True)
                nc.tensor.matmul(pi_[:], wr, si[:], start=True, stop=False,
                                 skip_group_check=True)
                nc.tensor.matmul(pr[:], wni, si[:], start=False, stop=True,
                                 skip_group_check=True)
                nc.tensor.matmul(pi_[:], wi, sr[:], start=False, stop=True,
                                 skip_group_check=True)
                return pr, pi_

            def mm_split(wa, wb, src, g):
                w_ = GROUPS[g][1] - GROUPS[g][0]
                pr = ps.tile([128, w_, 64], F32, name="psr", bufs=3)
                pi_ = ps.tile([128, w_, 64], F32, name="psi", bufs=3)
                sr, si = src
                for widx, hs in ((wa, slice(0, 32)), (wb, slice(32, 64))):
                    wr = wt[:, widx, :]
                    wni = wt[:, widx + 1, :]
                    wi = wt[:, widx + 2, :]
                    nc.tensor.matmul(pr[:, :, hs], wr, sr[:, :, hs],
                                     start=True, stop=False,
                                     skip_group_check=True)
                    nc.tensor.matmul(pi_[:, :, hs], wr, si[:, :, hs],
                                     start=True, stop=False,
                                     skip_group_check=True)
                    nc.tensor.matmul(pr[:, :, hs], wni, si[:, :, hs],
                                     start=False, stop=True,
                                     skip_group_check=True)
                    nc.tensor.matmul(pi_[:, :, hs], wi, sr[:, :, hs],
                                     start=False, stop=True,
                                     skip_group_check=True)
                return pr, pi_

            def fused_stage(widx, src, g):
                """fused lo-matmul + transpose: out_blk = S_blk.T @ bd(W);
                regular matmuls with the state block as stationary, ordered
                to share each stationary load across two matmuls."""
                w_ = GROUPS[g][1] - GROUPS[g][0]
                pr = ps.tile([128, w_, 64], F32, name="psr", bufs=3)
                pi_ = ps.tile([128, w_, 64], F32, name="psi", bufs=3)
                rr = wt[:, widx, :]
                rni = wt[:, widx + 1, :]
                ri = wt[:, widx + 2, :]
                sr, si = src
                for k in range(w_ // 2):
                    bs = slice(2 * k, 2 * k + 2)
                    nc.tensor.matmul(pr[:, bs, :], sr[:, bs, :], rr,
                                     start=True, stop=False,
                                     skip_group_check=True)
                    nc.tensor.matmul(pi_[:, bs, :], sr[:, bs, :], ri,
                                     start=True, stop=False,
                                     skip_group_check=True)
                    nc.tensor.matmul(pi_[:, bs, :], si[:, bs, :], rr,
                                     start=False, stop=True,
                                     skip_group_check=True)
                    nc.tensor.matmul(pr[:, bs, :], si[:, bs, :], rni,
                                     start=False, stop=True,
                                     skip_group_check=True)
                return pr, pi_

            def tr_stage(src, g):
                w_ = GROUPS[g][1] - GROUPS[g][0]
                pr = ps.tile([128, w_, 64], BF16, name="ptr", bufs=1)
                pi_ = ps.tile([128, w_, 64], BF16, name="pti", bufs=1)
                sr, si = src
                for k in range(w_ // 2):
                    bs = slice(2 * k, 2 * k + 2)
                    nc.tensor.transpose(pr[:, bs, :], sr[:, bs, :], ident)
                    nc.tensor.transpose(pi_[:, bs, :], si[:, bs, :], ident)
                return pr, pi_

            ev_engines = [
                lambda d, s: nc.vector.tensor_copy(d, s),
                lambda d, s: nc.scalar.copy(d, s),
            ]
            ev_ctr = [0]

            def evict(pr, pi_, name, g):
                w_ = GROUPS[g][1] - GROUPS[g][0]
                sr = sp.tile([128, w_, 64], BF16, name=f"s{name}r{g}", bufs=2)
                si = sp.tile([128, w_, 64], BF16, name=f"s{name}i{g}", bufs=2)
                ev_engines[ev_ctr[0] % 2](sr[:], pr[:])
                ev_engines[(ev_ctr[0] + 1) % 2](si[:], pi_[:])
                ev_ctr[0] += 1
                return sr, si

            plan = [
                ("fu", _W_F1, "1"),
                ("sp", (_W_P23A, _W_P23B), "23"),
                ("tr", None, "t2"),
                ("sp", (_W_P45A, _W_P45B), "45"),
                ("tr", None, "t3"),
                ("mm", _W_P6, "6"),
            ]

            def evict_pool(pr, pi_, name, g):
                w_ = GROUPS[g][1] - GROUPS[g][0]
                sr = sp.tile([128, w_, 64], BF16, name=f"s{name}r{g}", bufs=2)
                si = sp.tile([128, w_, 64], BF16, name=f"s{name}i{g}", bufs=2)
                nc.gpsimd.tensor_copy(sr[:], pr[:])
                nc.gpsimd.tensor_copy(si[:], pi_[:])
                return sr, si

            def emit_stage(s, src, g):
                kind, wi_, nm = plan[s]
                if kind == "mm":
                    pr, pi_ = mm_stage(wi_, src, g)
                elif kind == "sp":
                    pr, pi_ = mm_split(wi_[0], wi_[1], src, g)
                elif kind == "fu":
                    pr, pi_ = fused_stage(wi_, src, g)
                else:
                    pr, pi_ = tr_stage(src, g)
                return evict(pr, pi_, nm, g)

            # modulo-scheduled rep loop: at tick t, stage s runs rep t-s, so
            # adjacent engine-queue entries belong to independent chains and
            # every instruction's deps resolve before it reaches the head.
            cur = {}
            for tick in range(repeats + NST - 1):
                if tick < repeats:
                    pin = [[sp.tile([128, GROUPS[g][1] - GROUPS[g][0], 64],
                                    BF16, name=f"pin{pl}{g}", bufs=3)
                            for g in range(NG)] for pl in range(2)]
                    for g in range(NG):
                        a_, b_ = GROUPS[g]
                        for pl in range(2):
                            nc.sync.dma_start(pin[pl][g][:], p_d[pl][:, a_:b_, :])
                    for g in range(NG):
                        cur[(tick, g)] = (pin[0][g], pin[1][g])
                for s in range(NST):
                    rep = tick - s
                    if 0 <= rep < repeats:
                        for g in range(NG):
                            cur[(rep, g)] = emit_stage(s, cur[(rep, g)], g)
                rep_out = tick - (NST - 1)
                if rep_out >= 0:
                    for g in range(NG):
                        a_, b_ = GROUPS[g]
                        for pl in range(2):
                            nc.sync.dma_start(o_d[pl][:, a_:b_, :],
                                              cur[(rep_out, g)][pl][:])
                    for g in range(NG):
                        del cur[(rep_out, g)]
    nc.compile()
    return nc


# ---------------- host pre/post ----------------

def _prepare_in_maps(x, weights, entanglers):
    bf16 = ml_dtypes.bfloat16
    Ws, mask = _stage_matrices(weights, entanglers)
    W = _pack_w(Ws).astype(bf16)
    psi2 = _encode_psi2(x, weights, entanglers)
    in_maps = []
    for g in range(N_CORES):
        P = _pack_psi(psi2[BPC * g:BPC * (g + 1)]).astype(bf16)
        in_maps.append({"p": P, "w": W})
    return in_maps


def _postprocess(results, entanglers):
    mask = np.asarray(entanglers) > 0.5
    perm3 = _ring_perm(mask, 3)
    bits = (np.arange(DIM)[:, None] >> np.arange(N - 1, -1, -1)[None, :]) & 1
    signs = (1 - 2 * bits).astype(np.float32)
    slot = _unpack_slot()
    out = np.empty((BATCH, N), np.float32)
    for g in range(N_CORES):
        O = np.asarray(results[g]["o"], dtype=np.float32)  # [2, 128, 16, 64]
        P = O
        psi4 = np.empty((BPC, DIM), np.complex64)
        Pc = (P[0] + 1j * P[1]).reshape(2, 64, 16, 64)
        for g2 in range(2):
            for g16 in range(16):
                psi4[slot[g2, g16]] = Pc[g2, :, g16, :].reshape(DIM)
        psif = psi4[:, perm3]
        probs = (psif.real ** 2 + psif.imag ** 2).astype(np.float32)
        norm = probs.sum(axis=1, keepdims=True)
        out[BPC * g:BPC * (g + 1)] = (probs / norm) @ signs
    return out


def kernel(x, weights, entanglers):
    global LAST_EXEC_NS, LAST_RESULTS
    key = (np.asarray(x).tobytes(), np.asarray(weights).tobytes(),
           np.asarray(entanglers).tobytes())
    cached = _NC_CACHE.get("in_maps")
    if cached is not None and cached[0] == key:
        in_maps = cached[1]
    else:
        in_maps = _prepare_in_maps(x, weights, entanglers)
        _NC_CACHE["in_maps"] = (key, in_maps)

    if "nc" not in _NC_CACHE:
        _NC_CACHE["nc"] = _build_nc()
    nc = _NC_CACHE["nc"]

    trace = bool(os.environ.get("KERNEL_TRACE"))
    try:
        res = run_bass_kernel_spmd(nc, in_maps, core_ids=list(range(N_CORES)),
                                   trace=trace)
    except ModuleNotFoundError:
        res = run_bass_kernel_spmd(nc, in_maps, core_ids=list(range(N_CORES)),
                                   trace=False)
    LAST_RESULTS = res
    LAST_EXEC_NS = res.exec_time_ns
    return _postprocess(res.results, entanglers)
